# revision 6
# baseline (speedup 1.0000x reference)
"""LensCrackFault Trainium2 kernel.

out = clip(where(line_mask, 0.05, x), 0, 1) for x [32,3,512,512] f32 and
6 Bresenham lines per batch image given by endpoints [32,6,4] (y0,x0,y1,x1).

Strategy: the rasterization itself is tiny (192 lines x <=512 steps) and is
computed on host into a per-image uint8 mask. The device kernel is a pure
memory-streaming pass, data-parallel over the batch axis across 8 cores
(4 images per core): load image (3 channels, 3 MiB) -> overwrite crack
pixels with 0.05 via DVE copy_predicated (u8 mask, f32 const data) ->
store. Memory traffic per core: 12 MiB x read + 1 MiB mask read + 12 MiB
out write ~= 4% above the pure-copy roofline.
"""

import sys

sys.path.insert(0, "/opt/trn_rl_repo")

import numpy as np

import concourse.bacc as bacc
import concourse.mybir as mybir
from concourse import tile
from concourse.bass_utils import run_bass_kernel_spmd

N_CORES = 8
B, C, H, W = 32, 3, 512, 512
B_LOC = B // N_CORES  # 4 images per core
LINES_PER_IMG = 6
CRACK_VAL = 0.05
P = 128  # SBUF partitions
RPP = H // P  # image rows per partition (4)
FREE = RPP * W  # free-dim elems per partition per channel (2048)

_CACHE = {}


def rasterize_mask_np(endpoints: np.ndarray) -> np.ndarray:
    """Vectorized numpy port of the reference Bresenham scan -> u8 [B,H,W]."""
    ep = endpoints.reshape(-1, 4).astype(np.int64)
    y0, x0, y1, x1 = ep[:, 0], ep[:, 1], ep[:, 2], ep[:, 3]
    dx = np.abs(x1 - x0)
    dy = np.abs(y1 - y0)
    sx = np.where(x0 < x1, 1, -1)
    sy = np.where(y0 < y1, 1, -1)
    nsteps = np.maximum(dx, dy)
    cx = x0.copy()
    cy = y0.copy()
    err = dx - dy
    mask = np.zeros((B, H, W), dtype=np.uint8)
    b_idx = np.repeat(np.arange(B), LINES_PER_IMG)
    live = np.ones(ep.shape[0], dtype=bool)
    for t in range(max(H, W)):
        if not live.any():
            break
        mask[b_idx[live], cy[live], cx[live]] = 1
        e2 = 2 * err
        c1 = e2 > -dy
        c2 = e2 < dx
        err = err - np.where(c1, dy, 0) + np.where(c2, dx, 0)
        cx = cx + np.where(c1 & live, sx, 0)
        cy = cy + np.where(c2 & live, sy, 0)
        live = live & (t < nsteps)
    # The reference routes inactive scan steps to index (-1,-1), and jnp's
    # .at[].set wraps negative indices, so any image with a line shorter
    # than T-1 steps gets pixel (H-1, W-1) set.
    short = nsteps < max(H, W) - 1
    mask[b_idx[short], H - 1, W - 1] = 1
    return mask


def _build_nc():
    nc = bacc.Bacc("TRN2", target_bir_lowering=False, debug=False)
    x = nc.dram_tensor("x", [B_LOC, C, H, W], mybir.dt.float32, kind="ExternalInput")
    mask = nc.dram_tensor("mask", [B_LOC, H, W], mybir.dt.uint8, kind="ExternalInput")
    out = nc.dram_tensor("out", [B_LOC, C, H, W], mybir.dt.float32, kind="ExternalOutput")

    # DRAM views: partition dim over row-groups of RPP consecutive rows.
    # tile[p, c*FREE + q*W + w] = x[b, c, RPP*p + q, w]
    x_v = x.ap().rearrange("b c (p q) w -> b p c q w", p=P)
    o_v = out.ap().rearrange("b c (p q) w -> b p c q w", p=P)
    m_v = mask.ap().rearrange("b (p q) w -> b p q w", p=P)

    with tile.TileContext(nc) as tc:
        with (
            tc.tile_pool(name="const", bufs=1) as cpool,
            tc.tile_pool(name="work", bufs=3) as wpool,
            tc.tile_pool(name="mpool", bufs=3) as mpool,
        ):
            crack = cpool.tile([P, FREE], mybir.dt.float32)
            nc.vector.memset(crack[:], CRACK_VAL)
            for b in range(B_LOC):
                xt = wpool.tile([P, C * FREE], mybir.dt.float32, tag="xt")
                mt = mpool.tile([P, FREE], mybir.dt.uint8, tag="mt")
                xt_v = xt[:].rearrange("p (c q w) -> p c q w", c=C, q=RPP)
                nc.sync.dma_start(
                    out=mt[:].rearrange("p (q w) -> p q w", q=RPP), in_=m_v[b]
                )
                nc.sync.dma_start(out=xt_v, in_=x_v[b])
                for c in range(C):
                    nc.vector.copy_predicated(
                        xt[:, c * FREE : (c + 1) * FREE], mt[:], crack[:]
                    )
                nc.sync.dma_start(out=o_v[b], in_=xt_v)
    nc.compile()
    return nc


def _get_nc():
    if "nc" not in _CACHE:
        _CACHE["nc"] = _build_nc()
    return _CACHE["nc"]


def kernel(x, endpoints):
    x = np.ascontiguousarray(np.asarray(x, dtype=np.float32))
    endpoints = np.asarray(endpoints, dtype=np.int32)
    assert x.shape == (B, C, H, W), x.shape
    assert endpoints.shape == (B, LINES_PER_IMG, 4), endpoints.shape

    mask = rasterize_mask_np(endpoints)

    nc = _get_nc()
    in_maps = [
        {
            "x": x[i * B_LOC : (i + 1) * B_LOC],
            "mask": mask[i * B_LOC : (i + 1) * B_LOC],
        }
        for i in range(N_CORES)
    ]
    res = run_bass_kernel_spmd(nc, in_maps, core_ids=list(range(N_CORES)))
    out = np.concatenate([res.results[i]["out"] for i in range(N_CORES)], axis=0)
    return out


# revision 9
# speedup vs baseline: 1.1511x; 1.1511x over previous
"""LensCrackFault Trainium2 kernel.

out = clip(where(line_mask, 0.05, x), 0, 1) for x [32,3,512,512] f32 and
6 Bresenham lines per batch image given by endpoints [32,6,4] (y0,x0,y1,x1).

Strategy: the rasterization itself is tiny (192 lines x <=512 steps) and is
computed on host into a per-image uint8 mask. The device kernel is a pure
memory-streaming pass, data-parallel over the batch axis across 8 cores
(4 images per core): load image (3 channels, 3 MiB) -> overwrite crack
pixels with 0.05 via DVE copy_predicated (u8 mask, f32 const data) ->
store. Memory traffic per core: 12 MiB x read + 1 MiB mask read + 12 MiB
out write ~= 4% above the pure-copy roofline.
"""

import sys

sys.path.insert(0, "/opt/trn_rl_repo")

import numpy as np

import concourse.bacc as bacc
import concourse.mybir as mybir
from concourse import tile
from concourse.bass_utils import run_bass_kernel_spmd

N_CORES = 8
B, C, H, W = 32, 3, 512, 512
B_LOC = B // N_CORES  # 4 images per core
LINES_PER_IMG = 6
CRACK_VAL = 0.05
P = 128  # SBUF partitions
RPP = H // P  # image rows per partition (4)
FREE = RPP * W  # free-dim elems per partition per channel (2048)

_CACHE = {}


def rasterize_mask_np(endpoints: np.ndarray) -> np.ndarray:
    """Vectorized numpy port of the reference Bresenham scan -> u8 [B,H,W]."""
    ep = endpoints.reshape(-1, 4).astype(np.int64)
    y0, x0, y1, x1 = ep[:, 0], ep[:, 1], ep[:, 2], ep[:, 3]
    dx = np.abs(x1 - x0)
    dy = np.abs(y1 - y0)
    sx = np.where(x0 < x1, 1, -1)
    sy = np.where(y0 < y1, 1, -1)
    nsteps = np.maximum(dx, dy)
    cx = x0.copy()
    cy = y0.copy()
    err = dx - dy
    mask = np.zeros((B, H, W), dtype=np.uint8)
    b_idx = np.repeat(np.arange(B), LINES_PER_IMG)
    live = np.ones(ep.shape[0], dtype=bool)
    for t in range(max(H, W)):
        if not live.any():
            break
        mask[b_idx[live], cy[live], cx[live]] = 1
        e2 = 2 * err
        c1 = e2 > -dy
        c2 = e2 < dx
        err = err - np.where(c1, dy, 0) + np.where(c2, dx, 0)
        cx = cx + np.where(c1 & live, sx, 0)
        cy = cy + np.where(c2 & live, sy, 0)
        live = live & (t < nsteps)
    # The reference routes inactive scan steps to index (-1,-1), and jnp's
    # .at[].set wraps negative indices, so any image with a line shorter
    # than T-1 steps gets pixel (H-1, W-1) set.
    short = nsteps < max(H, W) - 1
    mask[b_idx[short], H - 1, W - 1] = 1
    return mask


def _build_nc():
    nc = bacc.Bacc("TRN2", target_bir_lowering=False, debug=False)
    x = nc.dram_tensor("x", [B_LOC, C, H, W], mybir.dt.float32, kind="ExternalInput")
    mask = nc.dram_tensor("mask", [B_LOC, H, W], mybir.dt.uint8, kind="ExternalInput")
    out = nc.dram_tensor("out", [B_LOC, C, H, W], mybir.dt.float32, kind="ExternalOutput")

    # DRAM views: partition dim over row-groups of RPP consecutive rows.
    # tile[p, c*FREE + q*W + w] = x[b, c, RPP*p + q, w]
    x_v = x.ap().rearrange("b c (p q) w -> b c p q w", p=P)
    o_v = out.ap().rearrange("b c (p q) w -> b c p q w", p=P)
    m_v = mask.ap().rearrange("b (p q) w -> b p q w", p=P)

    with tile.TileContext(nc) as tc:
        with (
            tc.tile_pool(name="const", bufs=1) as cpool,
            tc.tile_pool(name="work", bufs=6) as wpool,
            tc.tile_pool(name="mpool", bufs=3) as mpool,
        ):
            crack = cpool.tile([P, FREE], mybir.dt.float32)
            nc.vector.memset(crack[:], CRACK_VAL)
            for b in range(B_LOC):
                mt = mpool.tile([P, FREE], mybir.dt.uint8, tag="mt")
                nc.sync.dma_start(
                    out=mt[:].rearrange("p (q w) -> p q w", q=RPP), in_=m_v[b]
                )
                for c in range(C):
                    # per-channel 1 MiB chunks; loads on sync (HWDGE ring 1),
                    # stores on scalar (HWDGE ring 2) so a store's sem wait
                    # never stalls the next load's issue.
                    xt = wpool.tile([P, FREE], mybir.dt.float32, tag="xt")
                    xt_v = xt[:].rearrange("p (q w) -> p q w", q=RPP)
                    nc.sync.dma_start(out=xt_v, in_=x_v[b, c])
                    nc.vector.copy_predicated(xt[:], mt[:], crack[:])
                    nc.scalar.dma_start(out=o_v[b, c], in_=xt_v)
    nc.compile()
    return nc


def _get_nc():
    if "nc" not in _CACHE:
        _CACHE["nc"] = _build_nc()
    return _CACHE["nc"]


def kernel(x, endpoints):
    x = np.ascontiguousarray(np.asarray(x, dtype=np.float32))
    endpoints = np.asarray(endpoints, dtype=np.int32)
    assert x.shape == (B, C, H, W), x.shape
    assert endpoints.shape == (B, LINES_PER_IMG, 4), endpoints.shape

    mask = rasterize_mask_np(endpoints)

    nc = _get_nc()
    in_maps = [
        {
            "x": x[i * B_LOC : (i + 1) * B_LOC],
            "mask": mask[i * B_LOC : (i + 1) * B_LOC],
        }
        for i in range(N_CORES)
    ]
    res = run_bass_kernel_spmd(nc, in_maps, core_ids=list(range(N_CORES)))
    out = np.concatenate([res.results[i]["out"] for i in range(N_CORES)], axis=0)
    return out
